# revision 1
# baseline (speedup 1.0000x reference)
"""NeuroPlasticLite Trainium2 kernel (8-core data-parallel over batch).

Math (per core, batch shard BS=64):
  rows r = (b, n), b in [0,64), n in [0,256).
  x-space layout: SBUF [128 partitions = nlo, free = fc*32 + d] where
  fc = nhi*64 + b, n = nhi*128 + nlo.  (p = nlo makes the activation
  tensor a[b, m] naturally contraction-partitioned for the syn matmul.)

  Loop (20 steps), x kept in SBUF, updated via
    x <- c1*x + PSUM(h-matmuls + V-pass)
  where h = DT*(gelu(w1*syn + b1) @ w2.T), V = DT*(u@w_in.T + bias + b2),
  c1 = 1 - DT*GAMMA.  syn = a @ W_sp with W_sp the dense scatter of the
  top-50 cosine-sim weights (computed host-side, replicated).
"""

import os
from contextlib import ExitStack

import numpy as np

N, D, KF, KN = 256, 32, 16, 50
GAMMA, LAM_A, DT, STEPS = 0.1, 0.95, 0.05, 20
B, UIN = 512, 128
NCORES = 8
BS = B // NCORES          # 64 batch rows per core
R = BS * N                # 16384 rows per core
C1 = 1.0 - DT * GAMMA     # 0.995
EPS = 1e-12

_cache = {}


def _host_prep(features, bias, w_in, b_in, sig_w1, sig_b1, sig_w2, sig_b2):
    """All tiny, replicated tensors, as numpy (fp32)."""
    f = features / np.linalg.norm(features, axis=1, keepdims=True)
    sim = f @ f.T                                   # [N, N]
    # top-KN per row (order irrelevant; ties vanishingly unlikely)
    idx = np.argsort(-sim, axis=1, kind="stable")[:, :KN]        # [N, KN]
    vals = np.take_along_axis(sim, idx, axis=1)                  # [N, KN]
    W = np.zeros((N, N), np.float32)                             # W[m, n]
    np.add.at(W, (idx, np.arange(N)[:, None]), vals)

    # syn-matmul lhsT blocks: wt[:, (mhi*2+nhi)*128 : +128][mlo, nlo]
    #   = W[mhi*128+mlo, nhi*128+nlo]
    wt = np.concatenate(
        [W[mh * 128:(mh + 1) * 128, nh * 128:(nh + 1) * 128]
         for mh in (0, 1) for nh in (0, 1)], axis=1,
    ).astype(np.float32)                                         # [128, 512]

    winT = (DT * w_in.T).astype(np.float32)                      # [128, 32]
    # u_proj uses u @ w_in.T + b_in ; b_in is part of the loop-invariant V
    # biasSm[p, nhi*32 + d] = DT*(bias[nhi*128+p, d] + b_in[d] + sig_b2[d])
    badd = bias + b_in[None, :] + sig_b2[None, :]                # [256, 32]
    biasSm = np.concatenate(
        [DT * badd[0:128, :], DT * badd[128:256, :]], axis=1
    ).astype(np.float32)                                         # [128, 64]

    # h-matmul rhs: block-diag bd[(a8, j16), (a'8, d32)] = d(a==a')*DT*w2[d, j]
    bd = np.zeros((128, 256), np.float32)
    for a in range(8):
        # rows a*16..a*16+16 (j), cols a*32..a*32+32 (d)
        bd[a * 16:(a + 1) * 16, a * 32:(a + 1) * 32] = DT * sig_w2.T
    # round bd to tf32 (f32r) so the f32r h-matmul consumes exact values
    bdi = bd.view(np.uint32)
    bdi &= np.uint32(0xFFFFE000)
    ident = np.eye(128, dtype=np.float32)

    w1 = [float(sig_w1[j, 0]) for j in range(16)]
    b1 = [float(sig_b1[j]) for j in range(16)]
    return wt, winT, biasSm, bd, ident, w1, b1


def build_nc(w1, b1, n_cores):
    import concourse.bacc as bacc
    import concourse.tile as tile
    from concourse import mybir

    f32 = mybir.dt.float32
    f32r = mybir.dt.float32r
    AF = mybir.ActivationFunctionType
    OP = mybir.AluOpType
    AX = mybir.AxisListType

    nc = bacc.Bacc("TRN2", target_bir_lowering=False, debug=False,
                   num_devices=n_cores)
    u_s = nc.declare_dram_parameter("u_s", [R, UIN], f32, isOutput=False)
    wt_d = nc.declare_dram_parameter("wt", [128, 512], f32, isOutput=False)
    winT_d = nc.declare_dram_parameter("winT", [128, 32], f32, isOutput=False)
    biasSm_d = nc.declare_dram_parameter("biasSm", [128, 64], f32, isOutput=False)
    bd_d = nc.declare_dram_parameter("bd", [128, 256], f32r, isOutput=False)
    ident_d = nc.declare_dram_parameter("ident", [128, 128], f32, isOutput=False)
    xout = nc.declare_dram_parameter("xout", [128, 4096], f32, isOutput=True)

    def r32(ap):
        return ap.bitcast(f32r)

    with tile.TileContext(nc) as tc:
        with ExitStack() as ctx:
            cpool = ctx.enter_context(tc.tile_pool(name="consts", bufs=1))
            wt = cpool.tile([128, 512], f32)
            nc.sync.dma_start(wt[:], wt_d[:])
            winT = cpool.tile([128, 32], f32)
            nc.sync.dma_start(winT[:], winT_d[:])
            biasSm = cpool.tile([128, 64], f32)
            nc.sync.dma_start(biasSm[:], biasSm_d[:])
            bd = cpool.tile([128, 256], f32r)
            nc.sync.dma_start(bd[:], bd_d[:])
            ident = cpool.tile([128, 128], f32)
            nc.sync.dma_start(ident[:], ident_d[:])
            eps_t = cpool.tile([128, 1], f32)
            nc.vector.memset(eps_t[:], EPS)

            spool = ctx.enter_context(tc.tile_pool(name="state", bufs=1))
            x_sb = spool.tile([128, 4096], f32)
            V_sb = spool.tile([128, 4096], f32)
            G_sb = spool.tile([128, 2048], f32)

            # ---------- Phase A: u_proj -> V ----------
            with ExitStack() as actx:
                upool = actx.enter_context(tc.tile_pool(name="u", bufs=3))
                utp = actx.enter_context(
                    tc.tile_pool(name="utp", bufs=3, space="PSUM"))
                utsp = actx.enter_context(tc.tile_pool(name="uts", bufs=4))
                vpp = actx.enter_context(
                    tc.tile_pool(name="vp", bufs=2, space="PSUM"))

                def assemble_V(bank, vb):
                    nhi = (bank * 16) // 64
                    bsl = biasSm[:, nhi * 32:(nhi + 1) * 32]
                    brd = bsl.unsqueeze(1).broadcast_to((128, 16, 32))
                    nc.vector.tensor_tensor(
                        V_sb[:, bank * 512:(bank + 1) * 512].rearrange(
                            "p (s d) -> p s d", d=32),
                        vb[:].rearrange("p (s d) -> p s d", d=32),
                        brd, op=OP.add)

                for g in range(4):                     # bank pair (g, 4+g)
                    vlo = vpp.tile([128, 512], f32, tag="vlo")
                    vhi = vpp.tile([128, 512], f32, tag="vhi")
                    for c in range(4 * g, 4 * g + 4):  # u chunks of 1024 rows
                        uch = upool.tile([128, 1024], f32)
                        nc.sync.dma_start(
                            uch[:].rearrange("p (s k) -> p s k", k=128),
                            u_s[1024 * c:1024 * (c + 1), :].rearrange(
                                "(s p) k -> p s k", p=128))
                        for sb in range(8):
                            i = 8 * c + sb
                            fc = (i % 2) * 64 + i // 2
                            vb = vlo if fc < 64 else vhi
                            slot = fc % 16
                            tp = utp.tile([128, 128], f32)
                            nc.tensor.transpose(
                                tp[:], uch[:, sb * 128:(sb + 1) * 128],
                                ident[:])
                            uts = utsp.tile([128, 128], f32)
                            nc.scalar.copy(uts[:], tp[:])
                            nc.tensor.matmul(
                                vb[:, slot * 32:(slot + 1) * 32], uts[:],
                                winT[:], start=True, stop=True)
                    assemble_V(g, vlo)
                    assemble_V(4 + g, vhi)

            nc.vector.memset(x_sb[:], 0.0)

            # ---------- Phase B: 20 steps ----------
            lpool = ctx.enter_context(tc.tile_pool(name="loop", bufs=2))
            spsum = ctx.enter_context(
                tc.tile_pool(name="spsum", bufs=1, space="PSUM"))
            tpsum = ctx.enter_context(
                tc.tile_pool(name="tpsum", bufs=2, space="PSUM"))
            xpsum = ctx.enter_context(
                tc.tile_pool(name="xpsum", bufs=1, space="PSUM"))
            tsp = ctx.enter_context(tc.tile_pool(name="ts", bufs=3))

            for t in range(STEPS):
                xsq = lpool.tile([128, 4096], f32, tag="xsq")
                nsq = lpool.tile([128, 128], f32, tag="nsq")
                for ch in range(4):
                    csl = slice(ch * 1024, (ch + 1) * 1024)
                    if ch % 2 == 0:
                        nc.scalar.activation(
                            xsq[:, csl], x_sb[:, csl], AF.Square)
                    else:
                        nc.vector.tensor_tensor(
                            xsq[:, csl], x_sb[:, csl], x_sb[:, csl],
                            op=OP.mult)
                    nc.vector.reduce_sum(
                        nsq[:, ch * 32:(ch + 1) * 32],
                        xsq[:, csl].rearrange("p (f d) -> p f d", d=32),
                        axis=AX.X)
                nrm = lpool.tile([128, 128], f32, tag="nrm")
                nc.scalar.activation(nrm[:], nsq[:], AF.Sqrt, bias=eps_t[:, 0:1])
                a_sb = lpool.tile([128, 128], f32, tag="a")
                nc.scalar.activation(a_sb[:], nrm[:], AF.Tanh)

                syn = spsum.tile([128, 128], f32, tag="syn")
                for nh in (0, 1):
                    for mh in (0, 1):
                        nc.tensor.matmul(
                            syn[:, nh * 64:(nh + 1) * 64],
                            wt[:, (mh * 2 + nh) * 128:(mh * 2 + nh + 1) * 128],
                            a_sb[:, mh * 64:(mh + 1) * 64],
                            start=(mh == 0), stop=(mh == 1))

                for j in range(16):
                    nc.scalar.activation(
                        G_sb[:, j:2048:16], syn[:], AF.Gelu,
                        bias=b1[j], scale=w1[j])

                for half in (0, 1):
                    xp = xpsum.tile([128, 2048], f32, tag="xp")
                    for c in range(4):
                        nc.tensor.matmul(
                            xp[:, c * 512:(c + 1) * 512], ident[:],
                            V_sb[:, half * 2048 + c * 512:
                                 half * 2048 + (c + 1) * 512],
                            start=True, stop=False, skip_group_check=True)
                    for q in (0, 1):                 # quads of 4 transposes
                        tp4 = tpsum.tile([128, 512], f32, tag="gt4")
                        for o4 in range(4):
                            O = half * 8 + q * 4 + o4
                            nc.tensor.transpose(
                                tp4[:, o4 * 128:(o4 + 1) * 128],
                                G_sb[:, O * 128:(O + 1) * 128], ident[:])
                        ts4 = tsp.tile([128, 512], f32r, tag="ts4")
                        nc.scalar.copy(ts4[:], tp4[:])
                        for o4 in range(4):
                            o = q * 4 + o4
                            nc.tensor.matmul(
                                xp[:, o * 256:(o + 1) * 256],
                                ts4[:, o4 * 128:(o4 + 1) * 128], bd[:],
                                start=False, stop=(o == 7),
                                skip_group_check=True)
                    sl = slice(half * 2048, (half + 1) * 2048)
                    nc.vector.scalar_tensor_tensor(
                        x_sb[:, sl], x_sb[:, sl], C1, xp[:],
                        op0=OP.mult, op1=OP.add)

            # ---------- Phase C: output ----------
            nc.sync.dma_start(xout[:], x_sb[:])
    nc.finalize()
    return nc


def _get_nc(key, w1, b1, n_cores):
    if key not in _cache:
        _cache[key] = build_nc(w1, b1, n_cores)
    return _cache[key]


def kernel(u, features, bias, w_in, b_in, sig_w1, sig_b1, sig_w2, sig_b2):
    from concourse.bass_utils import run_bass_kernel_spmd

    u = np.asarray(u, np.float32)
    args = [np.asarray(a, np.float32) for a in
            (features, bias, w_in, b_in, sig_w1, sig_b1, sig_w2, sig_b2)]
    wt, winT, biasSm, bd, ident, w1, b1 = _host_prep(*args)

    key = (tuple(w1), tuple(b1))
    nc = _get_nc(key, w1, b1, NCORES)

    in_maps = []
    for c in range(NCORES):
        u_shard = np.ascontiguousarray(
            u[c * BS:(c + 1) * BS].reshape(R, UIN))
        in_maps.append({
            "u_s": u_shard, "wt": wt, "winT": winT,
            "biasSm": biasSm, "bd": bd, "ident": ident,
        })
    res = run_bass_kernel_spmd(nc, in_maps, list(range(NCORES)))

    out = np.empty((B, N, D), np.float32)
    for c in range(NCORES):
        xo = res.results[c]["xout"]                  # [128, 4096]
        # xo[nlo, fc*32+d]; fc = nhi*64+b, n = nhi*128+nlo
        v = xo.reshape(128, 2, 64, 32)               # [nlo, nhi, b, d]
        out[c * BS:(c + 1) * BS] = (
            v.transpose(2, 1, 0, 3).reshape(BS, N, D))
    return out



# revision 3
# speedup vs baseline: 2.2844x; 2.2844x over previous
"""NeuroPlasticLite Trainium2 kernel (8-core data-parallel over batch).

Layout (per core, batch shard BS=64):
  row r = (b, n); n = nhi*128 + nlo; fc = b*2 + nhi  (b-major slots!)
  x_sb [128 part = nlo, free = fc*32 + d]  fp32 state
  V_bf [128, 4096] bf16 = DT*(u_proj + badd), same layout
  a_sb [128, 128] bf16, col = nhi*64 + b   ((nhi,b)-major for syn matmuls)
  G_sb [128, 2048] bf16, col = fc*16 + j
  ts   [128, 2048] bf16: 16 transposed G blocks; block m: ts[fcl*16+j, nlo]

Per step: norm^2 via squares + bf16 tree-reduce (DVE 2x), sqrt via
bf16-bit-trick seed (Act copies) + 2 Newton iters (DVE/Pool), tanh (Act),
syn matmuls (PE bf16), gelu 16x (Act, w1/b1 as scale/bias; optionally NJ
j's via PE-prepared psum), DMA-engine transposes, h + V accumulated in
PSUM via bd/ident matmuls (PE), x update stt (DVE/Pool). Zero activation
table reloads (Square/Tanh/Gelu/Copy all in gelu_and_others).
"""

import os
from contextlib import ExitStack

import numpy as np

N, D, KF, KN = 256, 32, 16, 50
GAMMA, LAM_A, DT, STEPS = 0.1, 0.95, 0.05, 20
B, UIN = 512, 128
NCORES = 8
BS = B // NCORES          # 64 batch rows per core
R = BS * N
C1 = 1.0 - DT * GAMMA     # 0.995

# --- tunables ---
NJ = 8          # j's computed via PE psum-prep (0..16); rest via Act gelu
SQA = 2048              # per group (of 2048): cols squared on Act; rest on DVE
NITER = STEPS - 1
SEED_C = 24375.5          # bf16 rsqrt bit-trick affine constant
NSQ_FLOOR = 1e-8
NR2 = False
PROBE = os.environ.get('K_PROBE', '')
REDUCE_ONE = False
STT_POOL = ()
TREE_POOL = int(os.environ.get("K_TREE_POOL", "2"))   # levels L(5-k)..L5 on Pool
NR1_POOL = bool(int(os.environ.get("K_NR1_POOL", "1")))
SQ_ACT = int(os.environ.get("K_SQ_ACT", "2048"))           # chunk indices (0..3) per group whose stt runs on Pool

_cache = {}


def _gelu(z):
    from scipy.special import erf
    return 0.5 * z * (1.0 + erf(z / np.sqrt(2.0)))


def _host_prep(features, bias, w_in, b_in, sig_w1, sig_b1, sig_w2, sig_b2):
    import ml_dtypes
    bf = ml_dtypes.bfloat16

    f = features / np.linalg.norm(features, axis=1, keepdims=True)
    sim = f @ f.T
    idx = np.argsort(-sim, axis=1, kind="stable")[:, :KN]
    vals = np.take_along_axis(sim, idx, axis=1)
    W = np.zeros((N, N), np.float32)
    np.add.at(W, (idx, np.arange(N)[:, None]), vals)

    # wt blocks (moving rhs of synT mms): wt[:, (mh*2+nh)*128+nlo][mlo]
    #   = W[mh*128+mlo, nh*128+nlo]
    wt = np.concatenate(
        [W[mh * 128:(mh + 1) * 128, nh * 128:(nh + 1) * 128]
         for mh in (0, 1) for nh in (0, 1)], axis=1).astype(bf)  # [128, 512]

    w1 = [float(sig_w1[j, 0]) for j in range(16)]
    b1 = [float(sig_b1[j]) for j in range(16)]

    # wselRep[q, v*128 + fcl*16+j] = ((q % 32) == v*8 + fcl) * w1[j]
    # (each 32-row variant replicated at every 32-partition base)
    wsel = np.zeros((128, 512), np.float32)
    for q in range(128):
        for v in range(4):
            for j in range(16):
                if (q % 32) // 8 == v:
                    fcl = (q % 32) % 8
                    wsel[q, v * 128 + fcl * 16 + j] = w1[j]
    # fix: value nonzero only when q%32 == v*8+fcl
    wsel[:] = 0
    for q in range(128):
        fcl = (q % 32) % 8
        v = (q % 32) // 8
        for j in range(16):
            wsel[q, v * 128 + fcl * 16 + j] = w1[j]
    wsel = wsel.astype(bf)
    # b1T[q] = b1[q % 16]  (gelu bias AP)
    b1T = np.asarray([b1[q % 16] for q in range(128)], np.float32)[:, None]

    winTb = (DT * w_in.T).astype(bf)                              # [128, 32]

    badd = bias + b_in[None, :] + sig_b2[None, :]                 # [256, 32]
    h0 = (_gelu(np.asarray(sig_b1, np.float32)) @ sig_w2.T).astype(np.float32)
    # biasV/biasX [128, 64]: [nlo, nh*32 + d]
    biasV = np.concatenate(
        [DT * badd[0:128, :], DT * badd[128:256, :]], axis=1).astype(np.float32)
    biasX = np.concatenate(
        [DT * (badd + h0)[0:128, :], DT * (badd + h0)[128:256, :]],
        axis=1).astype(np.float32)

    # bd [128, 256] bf16: row q = fcl*16 + j ; col = fcl'*32 + d
    bd = np.zeros((128, 256), np.float32)
    for fcl in range(8):
        for j in range(16):
            bd[fcl * 16 + j, fcl * 32:(fcl + 1) * 32] = DT * sig_w2[:, j]
    bd = bd.astype(bf)

    ident = np.eye(128, dtype=np.float32).astype(bf)
    return dict(wt=wt, wsel=wsel, b1T=b1T, winTb=winTb,
                biasV=biasV, biasX=biasX, bd=bd, ident=ident,
                w1=w1, b1=b1)


def build_nc(w1, b1, n_cores):
    import concourse.bacc as bacc
    import concourse.tile as tile
    from concourse import mybir

    f32 = mybir.dt.float32
    bf16 = mybir.dt.bfloat16
    i16 = mybir.dt.int16
    AF = mybir.ActivationFunctionType
    OP = mybir.AluOpType

    nc = bacc.Bacc("TRN2", target_bir_lowering=False, debug=False,
                   num_devices=n_cores)
    uT_d = nc.declare_dram_parameter("uT", [128, R], bf16, isOutput=False)
    wt_d = nc.declare_dram_parameter("wt", [128, 512], bf16, isOutput=False)
    wsel_d = nc.declare_dram_parameter("wsel", [128, 512], bf16, isOutput=False)
    b1T_d = nc.declare_dram_parameter("b1T", [128, 1], f32, isOutput=False)
    winTb_d = nc.declare_dram_parameter("winTb", [128, 32], bf16, isOutput=False)
    biasV_d = nc.declare_dram_parameter("biasV", [128, 64], f32, isOutput=False)
    biasX_d = nc.declare_dram_parameter("biasX", [128, 64], f32, isOutput=False)
    bd_d = nc.declare_dram_parameter("bd", [128, 256], bf16, isOutput=False)
    ident_d = nc.declare_dram_parameter("ident", [128, 128], bf16, isOutput=False)
    xout = nc.declare_dram_parameter("xout", [128, 4096], f32, isOutput=True)

    with tile.TileContext(nc) as tc:
        with ExitStack() as ctx:
            cpool = ctx.enter_context(tc.tile_pool(name="consts", bufs=1))
            wt = cpool.tile([128, 512], bf16)
            nc.sync.dma_start(wt[:], wt_d[:])
            wsel = cpool.tile([128, 512], bf16)
            nc.sync.dma_start(wsel[:], wsel_d[:])
            b1T = cpool.tile([128, 1], f32)
            nc.sync.dma_start(b1T[:], b1T_d[:])
            winTb = cpool.tile([128, 32], bf16)
            nc.sync.dma_start(winTb[:], winTb_d[:])
            biasV = cpool.tile([128, 64], f32)
            nc.sync.dma_start(biasV[:], biasV_d[:])
            biasX = cpool.tile([128, 64], f32)
            nc.sync.dma_start(biasX[:], biasX_d[:])
            bd = cpool.tile([128, 256], bf16)
            nc.sync.dma_start(bd[:], bd_d[:])
            ident = cpool.tile([128, 128], bf16)
            nc.sync.dma_start(ident[:], ident_d[:])

            spool = ctx.enter_context(tc.tile_pool(name="state", bufs=1))
            x_sb = spool.tile([128, 4096], f32)
            V_bf = spool.tile([128, 4096], bf16)

            # ---------- Phase A ----------
            # fc = nh*64 + b; chunk c = fc [c*16,(c+1)*16), nh = c // 4
            with ExitStack() as actx:
                upool = actx.enter_context(tc.tile_pool(name="u", bufs=3))
                vpp = actx.enter_context(
                    tc.tile_pool(name="vp", bufs=3, space="PSUM"))
                for c in range(8):
                    uch = upool.tile([128, 2048], bf16)
                    nc.sync.dma_start(uch[:], uT_d[:, c * 2048:(c + 1) * 2048])
                    vps = vpp.tile([128, 512], f32, tag="vps")
                    for sidx in range(16):
                        nc.tensor.matmul(
                            vps[:, sidx * 32:(sidx + 1) * 32],
                            uch[:, sidx * 128:(sidx + 1) * 128], winTb[:],
                            start=True, stop=True, skip_group_check=True)
                    nh = c // 4
                    vv = vps[:].rearrange("p (s d) -> p s d", d=32)
                    bV = biasV[:, nh * 32:(nh + 1) * 32].unsqueeze(1)\
                        .broadcast_to((128, 16, 32))
                    bX = biasX[:, nh * 32:(nh + 1) * 32].unsqueeze(1)\
                        .broadcast_to((128, 16, 32))
                    Vv = V_bf[:, c * 512:(c + 1) * 512].rearrange(
                        "p (s d) -> p s d", d=32)
                    Xv = x_sb[:, c * 512:(c + 1) * 512].rearrange(
                        "p (s d) -> p s d", d=32)
                    nc.vector.tensor_tensor(Vv, vv, bV, op=OP.add)
                    nc.vector.tensor_tensor(Xv, vv, bX, op=OP.add)

            # ---------- loop pools ----------
            lp = ctx.enter_context(tc.tile_pool(name="loop", bufs=2))
            synp_pool = ctx.enter_context(
                tc.tile_pool(name="synp", bufs=2, space="PSUM"))
            tsp = ctx.enter_context(
                tc.tile_pool(name="tsp", bufs=2, space="PSUM"))
            xpp = ctx.enter_context(
                tc.tile_pool(name="xpp", bufs=2, space="PSUM"))

            # group g: x cols  [g*1024,(g+1)*1024) u [2048+g*1024, ...)
            def gcols(g):
                return (slice(g * 1024, (g + 1) * 1024),
                        slice(2048 + g * 1024, 2048 + (g + 1) * 1024))

            def sq_stage(t, g, T):
                for r, sl in enumerate(gcols(g)):
                    if PROBE == "sq":
                        sl = slice(sl.start, sl.start + 256)
                    nc.scalar.activation(
                        T["xsq"][:, sl], x_sb[:, sl], AF.Square)

            def tree_stage(t, g, T):
                for r, sl in enumerate(gcols(g)):
                    xv = T["xsq"][:, sl].rearrange("p (f d) -> p f d", d=32)
                    o1 = slice((2 * g + r) * 512, (2 * g + r + 1) * 512)
                    t1v = T["t1"][:, o1].rearrange("p (f d) -> p f d", d=16)
                    if PROBE == "tree":
                        t1v = T["t1"][:, o1.start:o1.start + 128].rearrange(
                            "p (f d) -> p f d", d=16)
                        xv = xv[:, 0:8, :]
                    nc.vector.tensor_tensor(
                        t1v, xv[:, :, 0:16], xv[:, :, 16:32], op=OP.add)
                    t1r = T["t1"][:, o1].rearrange("p (f d) -> p f d", d=16)
                    o2 = slice((2 * g + r) * 256, (2 * g + r + 1) * 256)
                    t2v = T["t2"][:, o2].rearrange("p (f d) -> p f d", d=8)
                    nc.vector.tensor_tensor(
                        t2v, t1r[:, :, 0:8], t1r[:, :, 8:16], op=OP.add)
                    t2r = T["t2"][:, o2].rearrange("p (f d) -> p f d", d=8)
                    o3 = slice((2 * g + r) * 128, (2 * g + r + 1) * 128)
                    t3v = T["t3"][:, o3].rearrange("p (f d) -> p f d", d=4)
                    nc.vector.tensor_tensor(
                        t3v, t2r[:, :, 0:4], t2r[:, :, 4:8], op=OP.add)
                    t3r = T["t3"][:, o3].rearrange("p (f d) -> p f d", d=4)
                    o4 = slice((2 * g + r) * 64, (2 * g + r + 1) * 64)
                    t4v = T["t4"][:, o4].rearrange("p (f d) -> p f d", d=2)
                    nc.vector.tensor_tensor(
                        t4v, t3r[:, :, 0:2], t3r[:, :, 2:4], op=OP.add)
                    t4r = T["t4"][:, o4].rearrange("p (f d) -> p f d", d=2)
                    nsl = slice(g * 64 + r * 32, g * 64 + (r + 1) * 32)
                    nc.vector.scalar_tensor_tensor(
                        T["nsq"][:, nsl], t4r[:, :, 0], NSQ_FLOOR,
                        t4r[:, :, 1], op0=OP.max, op1=OP.add)

            def norm_stage(t, g, r, T):
                enr = nc.gpsimd if (2 * g + r) % 2 == 0 else nc.vector
                nslc = slice(g * 64 + r * 32, g * 64 + (r + 1) * 32)
                enr.tensor_scalar(
                    T["fsd"][:, nslc], T["nsq"][:, nslc].bitcast(i16),
                    -0.5, SEED_C, op0=OP.mult, op1=OP.add)
                enr.tensor_copy(
                    T["y0"][:, nslc].bitcast(i16), T["fsd"][:, nslc])
                if PROBE == "nr":
                    enr.tensor_copy(T["y1"][:, nslc], T["y0"][:, nslc])
                    enr.tensor_tensor(
                        T["nrm"][:, nslc], T["nsq"][:, nslc], T["y1"][:, nslc],
                        op=OP.mult)
                    nc.scalar.activation(
                        T["a"][:, r * 64 + g * 32:r * 64 + g * 32 + 32],
                        T["nrm"][:, nslc], AF.Tanh)
                    synp = T["synp"]
                    for nh in (0, 1):
                        nc.tensor.matmul(
                            synp[nh * 64 + g * 32:nh * 64 + g * 32 + 32, :],
                            T["a"][:, r * 64 + g * 32:r * 64 + g * 32 + 32],
                            wt[:, (r * 2 + nh) * 128:(r * 2 + nh + 1) * 128],
                            start=(r == 0), stop=(r == 1),
                            skip_group_check=True,
                            tile_position=(0, nh * 64 + g * 32))
                    return
                enr.tensor_tensor(
                    T["p1"][:, nslc], T["y0"][:, nslc], T["y0"][:, nslc],
                    op=OP.mult)
                enr.tensor_tensor(
                    T["p2"][:, nslc], T["p1"][:, nslc], T["nsq"][:, nslc],
                    op=OP.mult)
                enr.tensor_scalar(
                    T["ww"][:, nslc], T["p2"][:, nslc], -0.5, 1.5,
                    op0=OP.mult, op1=OP.add)
                enr.tensor_tensor(
                    T["y1"][:, nslc], T["y0"][:, nslc], T["ww"][:, nslc],
                    op=OP.mult)
                enr.tensor_tensor(
                    T["nrm"][:, nslc], T["nsq"][:, nslc], T["y1"][:, nslc],
                    op=OP.mult)
                # tanh -> a[:, r*64 + g*32 : +32] (fc2 cols of this range)
                nc.scalar.activation(
                    T["a"][:, r * 64 + g * 32:r * 64 + g * 32 + 32],
                    T["nrm"][:, nslc], AF.Tanh)
                # synT accumulation for contraction-half mh == r
                synp = T["synp"]
                for nh in (0, 1):
                    nc.tensor.matmul(
                        synp[nh * 64 + g * 32:nh * 64 + g * 32 + 32, :],
                        T["a"][:, r * 64 + g * 32:r * 64 + g * 32 + 32],
                        wt[:, (r * 2 + nh) * 128:(r * 2 + nh + 1) * 128],
                        start=(r == 0), stop=(r == 1),
                        skip_group_check=True,
                        tile_position=(0, nh * 64 + g * 32))

            def prep_stage(t, g, T):
                synp = T["synp"]
                for nh in (0, 1):
                    pr = slice(nh * 64 + g * 32, nh * 64 + g * 32 + 32)
                    nc.scalar.copy(T["synT"][pr, :], synp[pr, :])

            def gelu_stage(t, g, T):
                # per half h (nh=h side): own psum tile -> gelu -> ts sbuf
                for h in (0, 1):
                    tsps = tsp.tile([128, 512], f32, tag="tsps", name="tsps")
                    for k in range(4):
                        m = 8 * h + g * 4 + k
                        r = m % 4
                        base = (m // 4) * 32
                        nc.tensor.matmul(
                            tsps[:, k * 128:(k + 1) * 128],
                            wsel[base:base + 32, r * 128:(r + 1) * 128],
                            T["synT"][base:base + 32, :],
                            start=True, stop=True, skip_group_check=True,
                            tile_position=(base, 0))
                    gw = 128 if PROBE == "gelu" else 512
                    nc.scalar.activation(
                        T["ts"][:, g * 1024 + h * 512:g * 1024 + h * 512 + gw],
                        tsps[:, 0:gw], AF.Gelu, bias=b1T[:])

            def tails_stage(t, g, T):
                # 2 chunks of 1024 x-cols (one per range)
                for r in (0, 1):
                    xc = 2048 * r + g * 1024
                    ti = g * 1024 + r * 512
                    xp = xpp.tile([128, 1024], f32, tag="xp", name="xp")
                    for h in (0, 1):
                        nc.tensor.matmul(
                            xp[:, h * 512:(h + 1) * 512], ident[:],
                            V_bf[:, xc + h * 512:xc + (h + 1) * 512],
                            start=True, stop=False, skip_group_check=True)
                    for q in range(4):
                        nc.tensor.matmul(
                            xp[:, q * 256:(q + 1) * 256],
                            T["ts"][:, ti + q * 128:ti + (q + 1) * 128], bd[:],
                            start=False, stop=(q == 3), skip_group_check=True)
                    w = 512 if PROBE == "stt" else 1024
                    nc.vector.scalar_tensor_tensor(
                        x_sb[:, xc:xc + w], x_sb[:, xc:xc + w], C1,
                        xp[:, 0:w], op0=OP.mult, op1=OP.add)

            for t in range(NITER):
                T = {}
                T["xsq"] = lp.tile([128, 4096], bf16, tag="xsq", name="xsq")
                T["t1"] = lp.tile([128, 2048], bf16, tag="t1", name="t1")
                T["t2"] = lp.tile([128, 1024], bf16, tag="t2", name="t2")
                T["t3"] = lp.tile([128, 512], bf16, tag="t3", name="t3")
                T["t4"] = lp.tile([128, 256], bf16, tag="t4", name="t4")
                T["nsq"] = lp.tile([128, 128], bf16, tag="nsq", name="nsq")
                T["fsd"] = lp.tile([128, 128], f32, tag="fsd", name="fsd")
                T["y0"] = lp.tile([128, 128], bf16, tag="y0", name="y0")
                T["p1"] = lp.tile([128, 128], bf16, tag="p1", name="p1")
                T["p2"] = lp.tile([128, 128], bf16, tag="p2", name="p2")
                T["ww"] = lp.tile([128, 128], bf16, tag="ww", name="ww")
                T["y1"] = lp.tile([128, 128], f32, tag="y1", name="y1")
                T["nrm"] = lp.tile([128, 128], f32, tag="nrm", name="nrm")
                T["a"] = lp.tile([128, 128], bf16, tag="a", name="a")
                T["synT"] = lp.tile([128, 128], bf16, tag="synT", name="synT")
                T["ts"] = lp.tile([128, 2048], bf16, tag="ts", name="ts")
                T["synp"] = synp_pool.tile([128, 128], f32, tag="sy",
                                           name="synp")

                if t > 0:
                    tails_stage(t - 1, 0, Tp)
                sq_stage(t, 0, T)
                if t > 0:
                    tails_stage(t - 1, 1, Tp)
                sq_stage(t, 1, T)
                tree_stage(t, 0, T)
                tree_stage(t, 1, T)
                norm_stage(t, 0, 0, T)
                norm_stage(t, 1, 0, T)
                norm_stage(t, 0, 1, T)
                norm_stage(t, 1, 1, T)
                prep_stage(t, 0, T)
                gelu_stage(t, 0, T)
                prep_stage(t, 1, T)
                gelu_stage(t, 1, T)
                Tp = T
            tails_stage(NITER - 1, 0, Tp)
            tails_stage(NITER - 1, 1, Tp)

            # ---------- output ----------
            nc.sync.dma_start(xout[:], x_sb[:])
    nc.finalize()
    return nc


def _get_nc(key, w1, b1, n_cores):
    if key not in _cache:
        _cache[key] = build_nc(w1, b1, n_cores)
    return _cache[key]


def kernel(u, features, bias, w_in, b_in, sig_w1, sig_b1, sig_w2, sig_b2):
    import ml_dtypes
    from concourse.bass_utils import run_bass_kernel_spmd

    bf = ml_dtypes.bfloat16
    u = np.asarray(u, np.float32)
    args = [np.asarray(a, np.float32) for a in
            (features, bias, w_in, b_in, sig_w1, sig_b1, sig_w2, sig_b2)]
    hp = _host_prep(*args)

    key = (tuple(hp["w1"]), tuple(hp["b1"]))
    nc = _get_nc(key, hp["w1"], hp["b1"], NCORES)

    in_maps = []
    for c in range(NCORES):
        ush = u[c * BS:(c + 1) * BS].reshape(BS, 2, 128, UIN)  # [b, nh, nlo, k]
        uT = np.ascontiguousarray(
            ush.transpose(3, 1, 0, 2).reshape(UIN, R)).astype(bf)
        m = {"uT": uT}
        for k in ("wt", "wsel", "b1T", "winTb", "biasV", "biasX",
                  "bd", "ident"):
            m[k] = hp[k]
        in_maps.append(m)

    kw = {}
    if os.environ.get("KERNEL_TRACE"):
        td = os.environ.get("KERNEL_TRACE_DIR")
        if td:
            os.makedirs(td, exist_ok=True)
        kw = dict(trace=True, tmpdir=td)
    res = run_bass_kernel_spmd(nc, in_maps, list(range(NCORES)), **kw)
    if os.environ.get("KERNEL_TRACE"):
        globals()["_last_hw_ns"] = res.exec_time_ns
        globals()["_last_trace"] = res.instructions_and_trace
        globals()["_last_profile_json"] = res.profile_json

    out = np.empty((B, N, D), np.float32)
    for c in range(NCORES):
        xo = res.results[c]["xout"]                   # [128, 4096]
        v = xo.reshape(128, 2, 64, 32)                # [nlo, nh, b, d]
        out[c * BS:(c + 1) * BS] = v.transpose(2, 1, 0, 3).reshape(BS, N, D)
    return out


_last_sim_ns = 281335  # TimelineSim cost-model estimate (per-core schedule)
